# revision 47
# baseline (speedup 1.0000x reference)
"""Trainium2 Bass kernel for the gr+sim distillation loss.

Reference math (per batch row i with label l, T=4, K=1000, D=2048):
    predict  = log_softmax(pred/T)
    sim      = weight[label] @ weight.T          -> row l of Gram G = W@W.T
    ts_row   = softmax(relu(G[l])^0.3 / 0.3)
    conf     = softmax(teacher/T)[l]
    gr       = conf at l, (1-conf)/(K-1) elsewhere
    t        = 0.5*gr + 0.5*ts_row
    loss     = T^2 * mean_i( sum_k t*(ln t - predict) )

Restructuring:
  * The per-row gather-matmul collapses to ONE Gram matrix G = W@W.T over the
    1000 classes (4.1 GFLOP total instead of 33.6 GFLOP), sharded across the 8
    cores: each core owns <=128 classes and computes only its G rows + their
    ts softmax (batch rows are bucketed by label onto the owning core, so no
    collectives are needed).
  * sum_k t = 1 folds log_softmax into the scalar logZ = ln sum exp(pred/T).
  * FAST path: writing ts~ for the ts row with its label entry zeroed and
    d = ts[l], the exact identity sum_k ts~ = 1-d gives
        sum_k ln(u' + 0.5 ts~)      = K ln u' + (1-d)/(2u') + r1
        0.5 sum_k ts~ ln(u'+0.5ts~) = 0.5 (1-d) ln u'      + r2
        0.5 sum_k ts~ pred          = r3
    with |r1|,|r2|,|r3| rigorously bounded by measured quantities
    ((1-d), max|pred| per row). For this problem's Gram the bounds are
    ~1e-9 of the loss - far below the reference's own fp32 rounding. The
    host VERIFIES the bound per call and falls back to the FULL kernel
    (which computes the gather + ln sweep + both dot products on device)
    if the data ever violates it.
  * Per-core device work (fast path): 16 Gram matmuls' worth of PE, the ts
    softmax chain, and three fused-accumulation sweeps over [rows, K]:
    Zt (ScalarE Exp+accum), Zp (ScalarE Exp+accum), S (VectorE accum).
  * Heavy inputs are host-packed so each stream is a few DMAs of 128 fat
    contiguous descriptors. The O(B) combine runs on host in float64.
"""

import sys

sys.path.insert(0, "/opt/trn_rl_repo")

from contextlib import ExitStack

import ml_dtypes
import numpy as np

import concourse.bass as bass
import concourse.bacc as bacc
import concourse.mybir as mybir
import concourse.tile as tile
from concourse.bass_utils import run_bass_kernel_spmd
from concourse.tile_rust import add_dep_helper

NCORES = 8
K = 1000
D = 2048
P = 128
NCH = D // P  # contraction chunks
TEMP = 4.0
POW = 0.3
TSA_W = 1008  # full path: K ts~ values, [K] = diag, pad

BF16 = mybir.dt.bfloat16
F32 = mybir.dt.float32
I32 = mybir.dt.int32
FP8 = mybir.dt.float8e4
AF = mybir.ActivationFunctionType
OP = mybir.AluOpType
NPBF16 = ml_dtypes.bfloat16
NPFP8 = mybir.dt.np(FP8)

# Guard threshold: absolute bound on the analytic-collapse error of the mean
# loss (reference loss is ~45; its own fp32 noise is ~1e-5).
GUARD_ABS = 1e-4

# All ACT functions this kernel uses live together in the
# "natural_log_exp_and_others" table set. The default table chooser maps each
# function to the first set containing it, which thrashes ACT_TABLE_LOAD
# (~2.7us per switch) when Exp and Ln alternate. Strip these functions from
# every other set (names/order/ids preserved) so exactly one load is emitted.
_ACT_COMBINED_SET = "natural_log_exp_and_others"
_ACT_PATCHED = False


def _patch_act_tables():
    global _ACT_PATCHED
    if _ACT_PATCHED:
        return
    _ACT_PATCHED = True
    funcs = {AF.Exp, AF.Ln, AF.Relu, AF.Copy, AF.Identity}
    orig = bacc.get_activation_tables

    def patched(arch):
        tables = orig(arch)
        assert _ACT_COMBINED_SET in tables
        assert funcs <= tables[_ACT_COMBINED_SET]
        for name in tables:
            if name != _ACT_COMBINED_SET:
                tables[name] = tables[name] - funcs
        return tables

    bacc.get_activation_tables = patched


def _new_nc():
    _patch_act_tables()
    return bacc.Bacc(
        "TRN2",
        debug=False,
        enable_asserts=False,
        target_bir_lowering=False,
        num_devices=NCORES,
    )


def _emit_input_loads(nc, sp, NT, handles):
    """DMA the packed inputs. Few, fat transfers: per-dma_start dispatch costs
    ~2.4us on HW. Streams (teacher head/rest, pred, d1h) ride the SP HWDGE
    queue in consumption order; weights ride the SWDGE (gpsimd) queue
    concurrently so PE is fed without starving ScalarE."""
    h_wt, h_wl, h_d1h, h_tea, h_pred = handles
    n0 = 2 if NT > 2 else 1

    te0 = sp.tile([P, n0, K], FP8, name="te0")
    nc.sync.dma_start(
        out=te0[:],
        in_=h_tea.ap()[:, 0:n0 * K].rearrange("p (a k) -> p a k", a=n0))
    wl_sb = sp.tile([P, NCH, P], FP8)
    nc.gpsimd.dma_start(
        out=wl_sb[:], in_=h_wl.ap().rearrange("p (a c) -> p a c", a=NCH))
    wt_sb = sp.tile([P, NCH, K], FP8)
    nc.gpsimd.dma_start(
        out=wt_sb[:], in_=h_wt.ap().rearrange("p (a k) -> p a k", a=NCH))
    te1 = sp.tile([P, NT - n0, K], FP8, name="te1")
    nc.sync.dma_start(
        out=te1[:],
        in_=h_tea.ap()[:, n0 * K:].rearrange("p (a k) -> p a k", a=NT - n0))
    pr_sb = sp.tile([P, NT, K], FP8)
    nc.sync.dma_start(
        out=pr_sb[:], in_=h_pred.ap().rearrange("p (a k) -> p a k", a=NT))
    d1h_sb = sp.tile([P, K], BF16)
    nc.sync.dma_start(out=d1h_sb[:], in_=h_d1h.ap())

    wt_pairs = [wt_sb[:, 2 * j:2 * j + 2, :] for j in range(NCH // 2)]
    te_sl = [te0[:, t, :] if t < n0 else te1[:, t - n0, :] for t in range(NT)]
    return wt_pairs, wl_sb, d1h_sb, te_sl, pr_sb


def _emit_gram_head(nc, sp, gp, pp, wt_pairs, wl_sb):
    """Gram matmuls -> relu -> ln -> ^0.3 logits (s3)."""
    KH = K // 2
    eps_sb = sp.tile([P, 1], F32)
    nc.vector.memset(eps_sb[:], 1e-30)
    r_sb = gp.tile([P, K], F32)
    pss = [
        pp.tile([P, KH], F32, name=f"gram_ps{nh}", tag=f"gram_ps{nh}")
        for nh in range(2)
    ]
    npairs = NCH // 2
    for j in range(npairs):
        # DoubleRow: each matmul contracts chunk pair (2j, 2j+1)
        for nh in range(2):
            nc.tensor.matmul(
                pss[nh][:],
                wl_sb[:, 2 * j:2 * j + 2, :],
                wt_pairs[j][:, :, nh * KH:(nh + 1) * KH],
                start=(j == 0),
                stop=(j == npairs - 1),
                perf_mode=mybir.MatmulPerfMode.DoubleRow,
            )
    for nh in range(2):
        # relu doubles as the PSUM->SBUF move; VectorE keeps it off the
        # saturated ScalarE queue
        nc.vector.tensor_scalar(
            r_sb[:, nh * KH:(nh + 1) * KH], pss[nh][:], 0.0, None, OP.max)
    lnr_sb = gp.tile([P, K], F32)
    nc.scalar.activation(lnr_sb[:], r_sb[:], AF.Ln, bias=eps_sb[:])
    s3_sb = gp.tile([P, K], F32)
    nc.scalar.activation(s3_sb[:], lnr_sb[:], AF.Exp, scale=POW)
    return s3_sb


def _emit_gram_tail(nc, gp, s3_sb, d1h_sb):
    """Row softmax pieces of the ts table: max-shifted exps, denominator, diag.

    Returns (ev_sb, dun_sb, rzs_sb).
    """
    m_sb = gp.tile([P, 1], F32)
    nc.vector.tensor_reduce(m_sb[:], s3_sb[:], axis=mybir.AxisListType.X, op=OP.max)
    negm_sb = gp.tile([P, 1], F32)
    nc.vector.tensor_scalar(negm_sb[:], m_sb[:], -1.0 / POW, None, OP.mult)
    ev_sb = gp.tile([P, K], F32)
    zs_sb = gp.tile([P, 1], F32)
    nc.scalar.activation(
        ev_sb[:], s3_sb[:], AF.Exp, bias=negm_sb[:], scale=1.0 / POW,
        accum_out=zs_sb[:],
    )
    rzs_sb = gp.tile([P, 1], F32)
    nc.vector.reciprocal(rzs_sb[:], zs_sb[:])
    gdump = gp.tile([P, K], BF16)
    dun_sb = gp.tile([P, 1], F32)
    nc.vector.scalar_tensor_tensor(
        out=gdump[:], in0=ev_sb[:], scalar=1.0, in1=d1h_sb[:],
        op0=OP.mult, op1=OP.mult, accum_out=dun_sb[:],
    )
    return ev_sb, dun_sb, rzs_sb


def build_nc_fast(NT: int, reps: int = 1):
    """Fast path: device computes Zt, Zp, S per row plus the per-class d.

    reps > 1 wraps the whole body in a hardware loop (benchmarking only).
    """
    nc = _new_nc()
    h_wt = nc.dram_tensor("wt", [P, NCH * K], FP8, kind="ExternalInput")
    h_wl = nc.dram_tensor("wl", [P, NCH * P], FP8, kind="ExternalInput")
    h_d1h = nc.dram_tensor("d1h", [P, K], BF16, kind="ExternalInput")
    h_tea = nc.dram_tensor("teab", [P, NT * K], FP8, kind="ExternalInput")
    h_pred = nc.dram_tensor("predb", [P, NT * K], FP8, kind="ExternalInput")
    h_oz = nc.dram_tensor("o_z", [P, 2 * NT], F32, kind="ExternalOutput")  # zt|zp
    h_osd = nc.dram_tensor("o_sd", [P, NT + 1], F32, kind="ExternalOutput")  # s|d

    with tile.TileContext(nc) as tc:
        with ExitStack() as ctx:
            sp = ctx.enter_context(tc.tile_pool(name="singles", bufs=1))
            gp = ctx.enter_context(tc.tile_pool(name="gram", bufs=1))
            pp = ctx.enter_context(tc.tile_pool(name="psum", bufs=2, space="PSUM"))
            du = ctx.enter_context(tc.tile_pool(name="dumps", bufs=2))
            if reps > 1:
                ctx.enter_context(tc.For_i(0, reps, 1))

            wt_pairs, wl_sb, d1h_sb, te_sl, pr_sb = _emit_input_loads(
                nc, sp, NT, (h_wt, h_wl, h_d1h, h_tea, h_pred))
            z_sb = sp.tile([P, 2 * NT], F32)
            sd_sb = sp.tile([P, NT + 1], F32)

            for t in range(NT):
                dm = du.tile([P, K], BF16, tag="dmT", name=f"dmT{t}")
                nc.scalar.activation(
                    dm[:], te_sl[t], AF.Exp,
                    scale=1.0 / TEMP, accum_out=z_sb[:, t:t + 1],
                )
            s3_sb = _emit_gram_head(nc, sp, gp, pp, wt_pairs, wl_sb)
            # diag-shifted softmax denominator: exp((s-s_diag)/0.3) sums to
            # Zd with the diagonal contributing exactly 1 -> d = 1/Zd. (If the
            # diagonal is not the row max on pathological data, Zd overflows
            # to inf, d -> 0, and the host guard falls back to the full path.)
            gdump = gp.tile([P, K], BF16)
            dg_sb = gp.tile([P, 1], F32)
            nc.vector.scalar_tensor_tensor(
                out=gdump[:], in0=s3_sb[:], scalar=1.0, in1=d1h_sb[:],
                op0=OP.mult, op1=OP.mult, accum_out=dg_sb[:],
            )
            negdg_sb = gp.tile([P, 1], F32)
            nc.vector.tensor_scalar(negdg_sb[:], dg_sb[:], -1.0 / POW, None, OP.mult)
            ev_sb = gp.tile([P, K], F32)
            zd_sb = gp.tile([P, 1], F32)
            for t in range(NT):
                d0 = du.tile([P, K], BF16, tag="d0", name=f"d0_{t}")
                nc.scalar.activation(
                    d0[:], pr_sb[:, t, :], AF.Exp, scale=1.0 / TEMP,
                    accum_out=z_sb[:, NT + t:NT + t + 1],
                )
                d3 = du.tile([P, K], BF16, tag="d3", name=f"d3_{t}")
                nc.vector.tensor_scalar(
                    d3[:], pr_sb[:, t, :], 1.0, None, OP.mult, OP.add,
                    accum_out=sd_sb[:, t:t + 1],
                )
                if t == 0:
                    nc.scalar.activation(
                        ev_sb[:], s3_sb[:], AF.Exp, bias=negdg_sb[:],
                        scale=1.0 / POW, accum_out=zd_sb[:],
                    )
            nc.vector.reciprocal(sd_sb[:, NT:NT + 1], zd_sb[:])
            nc.sync.dma_start(out=h_oz.ap(), in_=z_sb[:])
            nc.sync.dma_start(out=h_osd.ap(), in_=sd_sb[:])

    nc.compile()
    return nc


def build_nc_full(NT: int):
    """Full path: adds the ts~ row gather, the ln sweep, and both dots."""
    nc = _new_nc()
    h_wt = nc.dram_tensor("wt", [P, NCH * K], FP8, kind="ExternalInput")
    h_wl = nc.dram_tensor("wl", [P, NCH * P], FP8, kind="ExternalInput")
    h_d1h = nc.dram_tensor("d1h", [P, K], BF16, kind="ExternalInput")
    h_tea = nc.dram_tensor("teab", [P, NT * K], FP8, kind="ExternalInput")
    h_pred = nc.dram_tensor("predb", [P, NT * K], FP8, kind="ExternalInput")
    h_ridx = nc.dram_tensor("ridx", [P, NT], I32, kind="ExternalInput")
    h_tlv = nc.dram_tensor("tlv", [P, NT], F32, kind="ExternalInput")
    h_ops = nc.dram_tensor("o_ps", [P, 2 * NT], F32, kind="ExternalOutput")  # zp|slv
    h_ov = nc.dram_tensor("o_v", [P, 3 * NT], F32, kind="ExternalOutput")  # a|e1h|s
    h_ouc = nc.dram_tensor("o_uc", [P, 2 * NT], F32, kind="ExternalOutput")  # u2|conf
    h_od = nc.dram_tensor("o_d", [P, NT], F32, kind="ExternalOutput")
    h_tsa = nc.dram_tensor("tsa", [P, TSA_W], BF16)  # internal

    with tile.TileContext(nc) as tc:
        with ExitStack() as ctx:
            sp = ctx.enter_context(tc.tile_pool(name="singles", bufs=1))
            gp = ctx.enter_context(tc.tile_pool(name="gram", bufs=1))
            pp = ctx.enter_context(tc.tile_pool(name="psum", bufs=2, space="PSUM"))
            st = ctx.enter_context(tc.tile_pool(name="stream", bufs=3))
            du = ctx.enter_context(tc.tile_pool(name="dumps", bufs=2))

            wt_pairs, wl_sb, d1h_sb, te_sl, pr_sb = _emit_input_loads(
                nc, sp, NT, (h_wt, h_wl, h_d1h, h_tea, h_pred))
            ridx_sb = sp.tile([P, NT], I32)
            nc.sync.dma_start(out=ridx_sb[:], in_=h_ridx.ap())
            tlv_sb = sp.tile([P, NT], F32)
            nc.sync.dma_start(out=tlv_sb[:], in_=h_tlv.ap())

            zt_sb = sp.tile([P, NT], F32)
            ps_sb = sp.tile([P, 2 * NT], F32)  # zp | slv   (ACT-written)
            v_sb = sp.tile([P, 3 * NT], F32)  # a | e1h | s (DVE-written)
            uc_sb = sp.tile([P, 2 * NT], F32)  # u2 | conf  (DVE-written)
            dc_sb = sp.tile([P, NT], F32)  # diag       (GPSIMD-written)
            et_sb = sp.tile([P, NT], F32)
            rzt_sb = sp.tile([P, NT], F32)

            for t in range(NT):
                dm = du.tile([P, K], BF16, tag="dmT", name=f"dmT{t}")
                nc.scalar.activation(
                    dm[:], te_sl[t], AF.Exp,
                    scale=1.0 / TEMP, accum_out=zt_sb[:, t:t + 1],
                )

            s3_sb = _emit_gram_head(nc, sp, gp, pp, wt_pairs, wl_sb)
            ev_sb, dun_sb, rzs_sb = _emit_gram_tail(nc, gp, s3_sb, d1h_sb)
            ndun_sb = gp.tile([P, 1], F32)
            nc.vector.tensor_scalar(ndun_sb[:], dun_sb[:], -1.0, None, OP.mult)
            evnd_sb = gp.tile([P, K], F32)
            nc.vector.scalar_tensor_tensor(
                out=evnd_sb[:], in0=d1h_sb[:], scalar=ndun_sb[:], in1=ev_sb[:],
                op0=OP.mult, op1=OP.add,
            )
            tsa_sb = gp.tile([P, TSA_W], BF16)
            nc.vector.tensor_scalar(tsa_sb[:, 0:K], evnd_sb[:], rzs_sb[:], None, OP.mult)
            nc.vector.tensor_scalar(tsa_sb[:, K:K + 1], dun_sb[:], rzs_sb[:], None, OP.mult)
            nc.vector.memset(tsa_sb[:, K + 1:TSA_W], 0.0)
            w_tsa = nc.sync.dma_start(out=h_tsa.ap(), in_=tsa_sb[:])

            nc.scalar.activation(et_sb[:], tlv_sb[:], AF.Exp, scale=1.0 / TEMP)
            nc.vector.reciprocal(rzt_sb[:], zt_sb[:])
            nc.vector.tensor_tensor(
                out=uc_sb[:, NT:2 * NT], in0=et_sb[:], in1=rzt_sb[:], op=OP.mult)
            c = 1.0 / (2.0 * (K - 1))
            nc.vector.tensor_scalar(
                uc_sb[:, 0:NT], uc_sb[:, NT:2 * NT], -c, c, OP.mult, OP.add)

            for t in range(NT):
                tsg = st.tile([P, TSA_W], BF16, tag="tsg", name=f"tsg{t}")
                g = nc.gpsimd.indirect_dma_start(
                    out=tsg[:],
                    out_offset=None,
                    in_=h_tsa.ap(),
                    in_offset=bass.IndirectOffsetOnAxis(ap=ridx_sb[:, t:t + 1], axis=0),
                )
                add_dep_helper(g.ins, w_tsa.ins, True, "tsa table RAW")
                prt = pr_sb[:, t, :]
                lv = st.tile([P, K], BF16, tag="lv", name=f"lv{t}")
                d0 = du.tile([P, K], BF16, tag="d0", name=f"d0_{t}")
                nc.scalar.activation(
                    d0[:], prt, AF.Exp, scale=1.0 / TEMP,
                    accum_out=ps_sb[:, t:t + 1],
                )
                nc.scalar.activation(
                    lv[:], tsg[:, 0:K], AF.Ln, scale=0.5, bias=uc_sb[:, t:t + 1],
                    accum_out=ps_sb[:, NT + t:NT + t + 1],
                )
                d1 = du.tile([P, K], BF16, tag="d1", name=f"d1_{t}")
                nc.vector.scalar_tensor_tensor(
                    out=d1[:], in0=tsg[:, 0:K], scalar=0.5, in1=prt,
                    op0=OP.mult, op1=OP.mult,
                    accum_out=v_sb[:, NT + t:NT + t + 1],
                )
                d2 = du.tile([P, K], BF16, tag="d2", name=f"d2_{t}")
                nc.vector.scalar_tensor_tensor(
                    out=d2[:], in0=tsg[:, 0:K], scalar=0.5, in1=lv[:],
                    op0=OP.mult, op1=OP.mult,
                    accum_out=v_sb[:, t:t + 1],
                )
                d3 = du.tile([P, K], BF16, tag="d3", name=f"d3_{t}")
                nc.vector.tensor_scalar(
                    d3[:], prt, 1.0, None, OP.mult, OP.add,
                    accum_out=v_sb[:, 2 * NT + t:2 * NT + t + 1],
                )
                nc.gpsimd.tensor_copy(out=dc_sb[:, t:t + 1], in_=tsg[:, K:K + 1])

            nc.sync.dma_start(out=h_ops.ap(), in_=ps_sb[:])
            nc.sync.dma_start(out=h_ov.ap(), in_=v_sb[:])
            nc.sync.dma_start(out=h_ouc.ap(), in_=uc_sb[:])
            nc.sync.dma_start(out=h_od.ap(), in_=dc_sb[:])

    nc.compile()
    return nc


def plan_inputs(pred, teacher, weight, label):
    """Bucket rows by label, assign classes to cores (LPT), build per-core inputs."""
    pred = np.asarray(pred)
    teacher = np.asarray(teacher)
    weight = np.asarray(weight)
    lab = np.asarray(label).astype(np.int64)
    B = pred.shape[0]

    counts = np.bincount(lab, minlength=K)
    present = np.nonzero(counts)[0]
    order = present[np.argsort(-counts[present], kind="stable")]
    core_cls = [[] for _ in range(NCORES)]
    core_rows = [0] * NCORES
    for c in order:
        elig = [i for i in range(NCORES) if len(core_cls[i]) < P]
        i = min(elig, key=lambda j: (core_rows[j], len(core_cls[j])))
        core_cls[i].append(int(c))
        core_rows[i] += int(counts[c])
    NT = max(2, -(-max(core_rows) // P))
    NT += NT % 2  # even tile count (teacher stream is split in halves)
    BP = NT * P

    order_by_lab = np.argsort(lab, kind="stable")
    starts = np.zeros(K + 1, np.int64)
    np.cumsum(counts, out=starts[1:])

    wtT_bf = np.ascontiguousarray(weight.T).astype(NPFP8)  # [D, K]
    # packed layout: partition p's row is the concat over chunks a of
    # wtT[a*128 + p, :] -> each DMA is 128 fat contiguous descriptors
    wt_pack = np.ascontiguousarray(
        wtT_bf.reshape(D // P, P, K).transpose(1, 0, 2).reshape(P, (D // P) * K))

    def pack_rows(x2d):
        nt = x2d.shape[0] // P
        return np.ascontiguousarray(
            x2d.reshape(nt, P, -1).transpose(1, 0, 2).reshape(P, -1))

    in_maps, meta = [], []
    for ci in range(NCORES):
        cls = core_cls[ci] or [int(present[0])]  # degenerate: <8 distinct labels
        rows = (np.concatenate([order_by_lab[starts[c]:starts[c + 1]] for c in cls])
                if core_cls[ci] else np.zeros(0, np.int64))
        n = len(rows)
        assert n <= BP
        slot = (np.concatenate(
            [np.full(int(counts[c]), k, np.int32) for k, c in enumerate(cls)])
            if n else np.zeros(0, np.int32))

        predb = np.zeros((BP, K), NPFP8)
        predb[:n] = pred[rows].astype(NPFP8)
        teab = np.zeros((BP, K), NPFP8)
        teab[:n] = teacher[rows].astype(NPFP8)

        ridx = np.zeros((P, NT), np.int32)
        tlv = np.zeros((P, NT), np.float32)
        j = np.arange(n)
        ridx[j % P, j // P] = slot
        tlv[j % P, j // P] = teacher[rows, lab[rows]]
        plv = pred[rows, lab[rows]].astype(np.float64)
        maxp = np.abs(pred[rows]).max(axis=1).astype(np.float64)

        cls_pad = np.asarray(cls + [cls[0]] * (P - len(cls)), np.int64)
        wl = np.ascontiguousarray(wtT_bf[:, cls_pad])  # [D, P]
        wl_pack = np.ascontiguousarray(
            wl.reshape(D // P, P, P).transpose(1, 0, 2).reshape(P, (D // P) * P))
        d1h = np.zeros((P, K), NPBF16)
        d1h[np.arange(P), cls_pad] = 1.0

        in_maps.append({
            "wt": wt_pack, "wl": wl_pack, "d1h": d1h,
            "predb": pack_rows(predb), "teab": pack_rows(teab),
            "ridx": ridx, "tlv": tlv,
        })
        meta.append({"n": n, "plv": plv, "slot": slot, "tlv64": tlv.astype(np.float64),
                     "maxp": maxp})

    assert sum(m["n"] for m in meta) == B
    return {"NT": NT, "B": B, "in_maps": in_maps, "meta": meta}


FAST_KEYS = ("wt", "wl", "d1h", "predb", "teab")


def finish_fast(plan, results):
    """Host combine for the fast path. Returns (loss, error_bound)."""
    NT = plan["NT"]
    total = 0.0
    bound = 0.0
    for ci in range(NCORES):
        r, m = results[ci], plan["meta"][ci]
        n = m["n"]
        slot = m["slot"]

        def col(arr, comp):
            return arr[:, comp * NT:(comp + 1) * NT].astype(np.float64).T.reshape(-1)[:n]

        zt, zp = col(r["o_z"], 0), col(r["o_z"], 1)
        s = r["o_sd"][:, 0:NT].astype(np.float64).T.reshape(-1)[:n]
        dtab = r["o_sd"][:, NT].astype(np.float64)
        d = dtab[slot]
        tlv = m["tlv64"].T.reshape(-1)[:n]
        pl = m["plv"][:n]
        maxp = m["maxp"][:n]

        conf = np.exp(tlv / TEMP) / zt
        u2 = (1.0 - conf) / (2.0 * (K - 1))
        lnu2 = np.log(u2)
        eps = np.maximum(1.0 - d, 0.0)
        vb = 0.5 * conf + 0.5 * d

        H = (K - 1) * u2 * lnu2 + 0.5 * eps + 0.5 * lnu2 * eps + vb * np.log(vb)
        E = u2 * s + (vb - u2) * pl
        total += float(np.sum(H - E / TEMP + np.log(zp)))

        # rigorous remainder bounds, using f32 resolution slack on d
        epsr = eps + 2e-7
        b = (
            0.5 * epsr * maxp / TEMP          # dropped E1h
            + epsr * epsr / (8.0 * u2)        # Slv second order
            + epsr * epsr / (4.0 * u2) * 0.5  # A second order
            + 0.5 * np.abs(lnu2) * 2e-7       # d resolution in A term
        )
        bound += float(np.sum(b))
    loss = (TEMP * TEMP) * total / plan["B"]
    err = (TEMP * TEMP) * bound / plan["B"]
    return np.array(loss, dtype=np.float32), err


def finish_full(plan, results):
    """Host combine for the full path (float64)."""
    NT = plan["NT"]
    total = 0.0
    for ci in range(NCORES):
        r, m = results[ci], plan["meta"][ci]
        n = m["n"]

        def col(arr, comp):
            return arr[:, comp * NT:(comp + 1) * NT].astype(np.float64).T.reshape(-1)[:n]

        zp, slv = col(r["o_ps"], 0), col(r["o_ps"], 1)
        a, e1h, s = col(r["o_v"], 0), col(r["o_v"], 1), col(r["o_v"], 2)
        u2, conf = col(r["o_uc"], 0), col(r["o_uc"], 1)
        d = col(r["o_d"], 0)
        pl = m["plv"][:n]

        vb = 0.5 * conf + 0.5 * d
        H = u2 * slv + a - u2 * np.log(u2) + vb * np.log(vb)
        E = u2 * s + e1h + (vb - u2) * pl
        total += float(np.sum(H - E / TEMP + np.log(zp)))
    loss = (TEMP * TEMP) * total / plan["B"]
    return np.array(loss, dtype=np.float32)


_NC_CACHE = {}


def get_nc(NT: int, kind: str = "fast"):
    key = (kind, NT)
    if key not in _NC_CACHE:
        _NC_CACHE[key] = (build_nc_fast if kind == "fast" else build_nc_full)(NT)
    return _NC_CACHE[key]


def kernel(pred, teacher, weight, label):
    plan = plan_inputs(pred, teacher, weight, label)
    nc = get_nc(plan["NT"], "fast")
    fast_maps = [{k: im[k] for k in FAST_KEYS} for im in plan["in_maps"]]
    res = run_bass_kernel_spmd(nc, fast_maps, core_ids=list(range(NCORES)))
    loss, err = finish_fast(plan, res.results)
    if err <= GUARD_ABS:
        return loss
    # Data violates the diag-dominance bound: run the full on-device kernel.
    nc = get_nc(plan["NT"], "full")
    res = run_bass_kernel_spmd(nc, plan["in_maps"], core_ids=list(range(NCORES)))
    return finish_full(plan, res.results)


# revision 50
# speedup vs baseline: 1.0497x; 1.0497x over previous
"""Trainium2 Bass kernel for the gr+sim distillation loss.

Reference math (per batch row i with label l, T=4, K=1000, D=2048):
    predict  = log_softmax(pred/T)
    sim      = weight[label] @ weight.T          -> row l of Gram G = W@W.T
    ts_row   = softmax(relu(G[l])^0.3 / 0.3)
    conf     = softmax(teacher/T)[l]
    gr       = conf at l, (1-conf)/(K-1) elsewhere
    t        = 0.5*gr + 0.5*ts_row
    loss     = T^2 * mean_i( sum_k t*(ln t - predict) )

Restructuring:
  * The per-row gather-matmul collapses to ONE Gram matrix G = W@W.T over the
    1000 classes (4.1 GFLOP total instead of 33.6 GFLOP), sharded across the 8
    cores: each core owns <=128 classes and computes only its G rows + their
    ts softmax (batch rows are bucketed by label onto the owning core, so no
    collectives are needed).
  * sum_k t = 1 folds log_softmax into the scalar logZ = ln sum exp(pred/T).
  * FAST path: writing ts~ for the ts row with its label entry zeroed and
    d = ts[l], the exact identity sum_k ts~ = 1-d gives
        sum_k ln(u' + 0.5 ts~)      = K ln u' + (1-d)/(2u') + r1
        0.5 sum_k ts~ ln(u'+0.5ts~) = 0.5 (1-d) ln u'      + r2
        0.5 sum_k ts~ pred          = r3
    with |r1|,|r2|,|r3| rigorously bounded by measured quantities
    ((1-d), max|pred| per row). For this problem's Gram the bounds are
    ~1e-9 of the loss - far below the reference's own fp32 rounding. The
    host VERIFIES the bound per call and falls back to the FULL kernel
    (which computes the gather + ln sweep + both dot products on device)
    if the data ever violates it.
  * Per-core device work (fast path): 16 Gram matmuls' worth of PE, the ts
    softmax chain, and three fused-accumulation sweeps over [rows, K]:
    Zt (ScalarE Exp+accum), Zp (ScalarE Exp+accum), S (VectorE accum).
  * Heavy inputs are host-packed so each stream is a few DMAs of 128 fat
    contiguous descriptors. The O(B) combine runs on host in float64.
"""

import sys

sys.path.insert(0, "/opt/trn_rl_repo")

from contextlib import ExitStack

import ml_dtypes
import numpy as np

import concourse.bass as bass
import concourse.bacc as bacc
import concourse.mybir as mybir
import concourse.tile as tile
from concourse.bass_utils import run_bass_kernel_spmd
from concourse.tile_rust import add_dep_helper

NCORES = 8
K = 1000
D = 2048
P = 128
NCH = D // P  # contraction chunks
TEMP = 4.0
POW = 0.3
TSA_W = 1008  # full path: K ts~ values, [K] = diag, pad

BF16 = mybir.dt.bfloat16
F32 = mybir.dt.float32
I32 = mybir.dt.int32
FP8 = mybir.dt.float8e4
AF = mybir.ActivationFunctionType
OP = mybir.AluOpType
NPBF16 = ml_dtypes.bfloat16
NPFP8 = mybir.dt.np(FP8)

# Guard threshold: absolute bound on the analytic-collapse error of the mean
# loss (reference loss is ~45; its own fp32 noise is ~1e-5).
GUARD_ABS = 1e-4

# All ACT functions this kernel uses live together in the
# "natural_log_exp_and_others" table set. The default table chooser maps each
# function to the first set containing it, which thrashes ACT_TABLE_LOAD
# (~2.7us per switch) when Exp and Ln alternate. Strip these functions from
# every other set (names/order/ids preserved) so exactly one load is emitted.
_ACT_COMBINED_SET = "natural_log_exp_and_others"
_ACT_PATCHED = False


def _patch_act_tables():
    global _ACT_PATCHED
    if _ACT_PATCHED:
        return
    _ACT_PATCHED = True
    funcs = {AF.Exp, AF.Ln, AF.Relu, AF.Copy, AF.Identity}
    orig = bacc.get_activation_tables

    def patched(arch):
        tables = orig(arch)
        assert _ACT_COMBINED_SET in tables
        assert funcs <= tables[_ACT_COMBINED_SET]
        for name in tables:
            if name != _ACT_COMBINED_SET:
                tables[name] = tables[name] - funcs
        return tables

    bacc.get_activation_tables = patched


def _new_nc():
    _patch_act_tables()
    return bacc.Bacc(
        "TRN2",
        debug=False,
        enable_asserts=False,
        target_bir_lowering=False,
        num_devices=NCORES,
    )


def _emit_input_loads(nc, sp, NT, handles):
    """DMA the packed inputs. Few, fat transfers: per-dma_start dispatch costs
    ~2.4us on HW. Streams (teacher head/rest, pred, d1h) ride the SP HWDGE
    queue in consumption order; weights ride the SWDGE (gpsimd) queue
    concurrently so PE is fed without starving ScalarE."""
    h_wt, h_wl, h_tea, h_pred = handles
    n0 = 2 if NT > 2 else 1

    te0 = sp.tile([P, n0, K], FP8, name="te0")
    nc.sync.dma_start(
        out=te0[:],
        in_=h_tea.ap()[:, 0:n0 * K].rearrange("p (a k) -> p a k", a=n0))
    wl_sb = sp.tile([P, NCH, P], FP8)
    nc.gpsimd.dma_start(
        out=wl_sb[:], in_=h_wl.ap().rearrange("p (a c) -> p a c", a=NCH))
    wt_sb = sp.tile([P, NCH, K], FP8)
    nc.gpsimd.dma_start(
        out=wt_sb[:], in_=h_wt.ap().rearrange("p (a k) -> p a k", a=NCH))
    te1 = sp.tile([P, NT - n0, K], FP8, name="te1")
    nc.sync.dma_start(
        out=te1[:],
        in_=h_tea.ap()[:, n0 * K:].rearrange("p (a k) -> p a k", a=NT - n0))
    # pred stream carries the diag one-hot appended as a final K-segment
    prd_sb = sp.tile([P, (NT + 1) * K], FP8)
    nc.sync.dma_start(
        out=prd_sb[:].rearrange("p (a k) -> p a k", a=NT + 1),
        in_=h_pred.ap().rearrange("p (a k) -> p a k", a=NT + 1))

    wt_pairs = [wt_sb[:, 2 * j:2 * j + 2, :] for j in range(NCH // 2)]
    te_sl = [te0[:, t, :] if t < n0 else te1[:, t - n0, :] for t in range(NT)]
    pr_sl = [prd_sb[:, t * K:(t + 1) * K] for t in range(NT)]
    d1h_sb = prd_sb[:, NT * K:(NT + 1) * K]
    return wt_pairs, wl_sb, d1h_sb, te_sl, pr_sl


def _emit_gram_head(nc, sp, gp, pp, wt_pairs, wl_sb):
    """Gram matmuls -> relu -> ln -> ^0.3 logits (s3)."""
    KH = K // 2
    eps_sb = sp.tile([P, 1], F32)
    nc.vector.memset(eps_sb[:], 1e-30)
    r_sb = gp.tile([P, K], F32)
    pss = [
        pp.tile([P, KH], F32, name=f"gram_ps{nh}", tag=f"gram_ps{nh}")
        for nh in range(2)
    ]
    npairs = NCH // 2
    for j in range(npairs):
        # DoubleRow: each matmul contracts chunk pair (2j, 2j+1)
        for nh in range(2):
            nc.tensor.matmul(
                pss[nh][:],
                wl_sb[:, 2 * j:2 * j + 2, :],
                wt_pairs[j][:, :, nh * KH:(nh + 1) * KH],
                start=(j == 0),
                stop=(j == npairs - 1),
                perf_mode=mybir.MatmulPerfMode.DoubleRow,
            )
    for nh in range(2):
        # relu doubles as the PSUM->SBUF move; VectorE keeps it off the
        # saturated ScalarE queue
        nc.vector.tensor_scalar(
            r_sb[:, nh * KH:(nh + 1) * KH], pss[nh][:], 0.0, None, OP.max)
    lnr_sb = gp.tile([P, K], F32)
    nc.scalar.activation(lnr_sb[:], r_sb[:], AF.Ln, bias=eps_sb[:])
    s3_sb = gp.tile([P, K], F32)
    nc.scalar.activation(s3_sb[:], lnr_sb[:], AF.Exp, scale=POW)
    return s3_sb


def _emit_gram_tail(nc, gp, s3_sb, d1h_sb):
    """Row softmax pieces of the ts table: max-shifted exps, denominator, diag.

    Returns (ev_sb, dun_sb, rzs_sb).
    """
    m_sb = gp.tile([P, 1], F32)
    nc.vector.tensor_reduce(m_sb[:], s3_sb[:], axis=mybir.AxisListType.X, op=OP.max)
    negm_sb = gp.tile([P, 1], F32)
    nc.vector.tensor_scalar(negm_sb[:], m_sb[:], -1.0 / POW, None, OP.mult)
    ev_sb = gp.tile([P, K], F32)
    zs_sb = gp.tile([P, 1], F32)
    nc.scalar.activation(
        ev_sb[:], s3_sb[:], AF.Exp, bias=negm_sb[:], scale=1.0 / POW,
        accum_out=zs_sb[:],
    )
    rzs_sb = gp.tile([P, 1], F32)
    nc.vector.reciprocal(rzs_sb[:], zs_sb[:])
    gdump = gp.tile([P, K], BF16)
    dun_sb = gp.tile([P, 1], F32)
    nc.vector.scalar_tensor_tensor(
        out=gdump[:], in0=ev_sb[:], scalar=1.0, in1=d1h_sb[:],
        op0=OP.mult, op1=OP.mult, accum_out=dun_sb[:],
    )
    return ev_sb, dun_sb, rzs_sb


def build_nc_fast(NT: int, reps: int = 1):
    """Fast path: device computes Zt, Zp, S per row plus the per-class d.

    reps > 1 wraps the whole body in a hardware loop (benchmarking only).
    """
    nc = _new_nc()
    h_wt = nc.dram_tensor("wt", [P, NCH * K], FP8, kind="ExternalInput")
    h_wl = nc.dram_tensor("wl", [P, NCH * P], FP8, kind="ExternalInput")
    h_tea = nc.dram_tensor("teab", [P, NT * K], FP8, kind="ExternalInput")
    h_pred = nc.dram_tensor("predb", [P, (NT + 1) * K], FP8, kind="ExternalInput")
    h_oz = nc.dram_tensor("o_z", [P, 2 * NT], F32, kind="ExternalOutput")  # zt|zp
    h_osd = nc.dram_tensor("o_sd", [P, NT + 1], F32, kind="ExternalOutput")  # s|d

    with tile.TileContext(nc) as tc:
        with ExitStack() as ctx:
            sp = ctx.enter_context(tc.tile_pool(name="singles", bufs=1))
            gp = ctx.enter_context(tc.tile_pool(name="gram", bufs=1))
            pp = ctx.enter_context(tc.tile_pool(name="psum", bufs=2, space="PSUM"))
            du = ctx.enter_context(tc.tile_pool(name="dumps", bufs=2))
            if reps > 1:
                ctx.enter_context(tc.For_i(0, reps, 1))

            wt_pairs, wl_sb, d1h_sb, te_sl, pr_sl = _emit_input_loads(
                nc, sp, NT, (h_wt, h_wl, h_tea, h_pred))
            z_sb = sp.tile([P, 2 * NT], F32)
            sd_sb = sp.tile([P, NT + 1], F32)

            for t in range(NT):
                dm = du.tile([P, K], BF16, tag="dmT", name=f"dmT{t}")
                nc.scalar.activation(
                    dm[:], te_sl[t], AF.Exp,
                    scale=1.0 / TEMP, accum_out=z_sb[:, t:t + 1],
                )
            s3_sb = _emit_gram_head(nc, sp, gp, pp, wt_pairs, wl_sb)
            # diag-shifted softmax denominator: exp((s-s_diag)/0.3) sums to
            # Zd with the diagonal contributing exactly 1 -> d = 1/Zd. (If the
            # diagonal is not the row max on pathological data, Zd overflows
            # to inf, d -> 0, and the host guard falls back to the full path.)
            gdump = gp.tile([P, K], BF16)
            dg_sb = gp.tile([P, 1], F32)
            nc.vector.scalar_tensor_tensor(
                out=gdump[:], in0=s3_sb[:], scalar=1.0, in1=d1h_sb[:],
                op0=OP.mult, op1=OP.mult, accum_out=dg_sb[:],
            )
            negdg_sb = gp.tile([P, 1], F32)
            nc.vector.tensor_scalar(negdg_sb[:], dg_sb[:], -1.0 / POW, None, OP.mult)
            ev_sb = gp.tile([P, K], F32)
            zd_sb = gp.tile([P, 1], F32)
            for t in range(NT):
                d0 = du.tile([P, K], BF16, tag="d0", name=f"d0_{t}")
                nc.scalar.activation(
                    d0[:], pr_sl[t], AF.Exp, scale=1.0 / TEMP,
                    accum_out=z_sb[:, NT + t:NT + t + 1],
                )
                d3 = du.tile([P, K], BF16, tag="d3", name=f"d3_{t}")
                nc.vector.tensor_scalar(
                    d3[:], pr_sl[t], 1.0, None, OP.mult, OP.add,
                    accum_out=sd_sb[:, t:t + 1],
                )
                if t == 0:
                    nc.scalar.activation(
                        ev_sb[:], s3_sb[:], AF.Exp, bias=negdg_sb[:],
                        scale=1.0 / POW, accum_out=zd_sb[:],
                    )
            nc.vector.reciprocal(sd_sb[:, NT:NT + 1], zd_sb[:])
            nc.sync.dma_start(out=h_oz.ap(), in_=z_sb[:])
            nc.gpsimd.dma_start(out=h_osd.ap(), in_=sd_sb[:])

    nc.compile()
    return nc


def build_nc_full(NT: int):
    """Full path: adds the ts~ row gather, the ln sweep, and both dots."""
    nc = _new_nc()
    h_wt = nc.dram_tensor("wt", [P, NCH * K], FP8, kind="ExternalInput")
    h_wl = nc.dram_tensor("wl", [P, NCH * P], FP8, kind="ExternalInput")
    h_tea = nc.dram_tensor("teab", [P, NT * K], FP8, kind="ExternalInput")
    h_pred = nc.dram_tensor("predb", [P, (NT + 1) * K], FP8, kind="ExternalInput")
    h_ridx = nc.dram_tensor("ridx", [P, NT], I32, kind="ExternalInput")
    h_tlv = nc.dram_tensor("tlv", [P, NT], F32, kind="ExternalInput")
    h_ops = nc.dram_tensor("o_ps", [P, 2 * NT], F32, kind="ExternalOutput")  # zp|slv
    h_ov = nc.dram_tensor("o_v", [P, 3 * NT], F32, kind="ExternalOutput")  # a|e1h|s
    h_ouc = nc.dram_tensor("o_uc", [P, 2 * NT], F32, kind="ExternalOutput")  # u2|conf
    h_od = nc.dram_tensor("o_d", [P, NT], F32, kind="ExternalOutput")
    h_tsa = nc.dram_tensor("tsa", [P, TSA_W], BF16)  # internal

    with tile.TileContext(nc) as tc:
        with ExitStack() as ctx:
            sp = ctx.enter_context(tc.tile_pool(name="singles", bufs=1))
            gp = ctx.enter_context(tc.tile_pool(name="gram", bufs=1))
            pp = ctx.enter_context(tc.tile_pool(name="psum", bufs=2, space="PSUM"))
            st = ctx.enter_context(tc.tile_pool(name="stream", bufs=3))
            du = ctx.enter_context(tc.tile_pool(name="dumps", bufs=2))

            wt_pairs, wl_sb, d1h_sb, te_sl, pr_sl = _emit_input_loads(
                nc, sp, NT, (h_wt, h_wl, h_tea, h_pred))
            ridx_sb = sp.tile([P, NT], I32)
            nc.sync.dma_start(out=ridx_sb[:], in_=h_ridx.ap())
            tlv_sb = sp.tile([P, NT], F32)
            nc.sync.dma_start(out=tlv_sb[:], in_=h_tlv.ap())

            zt_sb = sp.tile([P, NT], F32)
            ps_sb = sp.tile([P, 2 * NT], F32)  # zp | slv   (ACT-written)
            v_sb = sp.tile([P, 3 * NT], F32)  # a | e1h | s (DVE-written)
            uc_sb = sp.tile([P, 2 * NT], F32)  # u2 | conf  (DVE-written)
            dc_sb = sp.tile([P, NT], F32)  # diag       (GPSIMD-written)
            et_sb = sp.tile([P, NT], F32)
            rzt_sb = sp.tile([P, NT], F32)

            for t in range(NT):
                dm = du.tile([P, K], BF16, tag="dmT", name=f"dmT{t}")
                nc.scalar.activation(
                    dm[:], te_sl[t], AF.Exp,
                    scale=1.0 / TEMP, accum_out=zt_sb[:, t:t + 1],
                )

            s3_sb = _emit_gram_head(nc, sp, gp, pp, wt_pairs, wl_sb)
            ev_sb, dun_sb, rzs_sb = _emit_gram_tail(nc, gp, s3_sb, d1h_sb)
            ndun_sb = gp.tile([P, 1], F32)
            nc.vector.tensor_scalar(ndun_sb[:], dun_sb[:], -1.0, None, OP.mult)
            evnd_sb = gp.tile([P, K], F32)
            nc.vector.scalar_tensor_tensor(
                out=evnd_sb[:], in0=d1h_sb[:], scalar=ndun_sb[:], in1=ev_sb[:],
                op0=OP.mult, op1=OP.add,
            )
            tsa_sb = gp.tile([P, TSA_W], BF16)
            nc.vector.tensor_scalar(tsa_sb[:, 0:K], evnd_sb[:], rzs_sb[:], None, OP.mult)
            nc.vector.tensor_scalar(tsa_sb[:, K:K + 1], dun_sb[:], rzs_sb[:], None, OP.mult)
            nc.vector.memset(tsa_sb[:, K + 1:TSA_W], 0.0)
            w_tsa = nc.sync.dma_start(out=h_tsa.ap(), in_=tsa_sb[:])

            nc.scalar.activation(et_sb[:], tlv_sb[:], AF.Exp, scale=1.0 / TEMP)
            nc.vector.reciprocal(rzt_sb[:], zt_sb[:])
            nc.vector.tensor_tensor(
                out=uc_sb[:, NT:2 * NT], in0=et_sb[:], in1=rzt_sb[:], op=OP.mult)
            c = 1.0 / (2.0 * (K - 1))
            nc.vector.tensor_scalar(
                uc_sb[:, 0:NT], uc_sb[:, NT:2 * NT], -c, c, OP.mult, OP.add)

            for t in range(NT):
                tsg = st.tile([P, TSA_W], BF16, tag="tsg", name=f"tsg{t}")
                g = nc.gpsimd.indirect_dma_start(
                    out=tsg[:],
                    out_offset=None,
                    in_=h_tsa.ap(),
                    in_offset=bass.IndirectOffsetOnAxis(ap=ridx_sb[:, t:t + 1], axis=0),
                )
                add_dep_helper(g.ins, w_tsa.ins, True, "tsa table RAW")
                prt = pr_sl[t]
                lv = st.tile([P, K], BF16, tag="lv", name=f"lv{t}")
                d0 = du.tile([P, K], BF16, tag="d0", name=f"d0_{t}")
                nc.scalar.activation(
                    d0[:], prt, AF.Exp, scale=1.0 / TEMP,
                    accum_out=ps_sb[:, t:t + 1],
                )
                nc.scalar.activation(
                    lv[:], tsg[:, 0:K], AF.Ln, scale=0.5, bias=uc_sb[:, t:t + 1],
                    accum_out=ps_sb[:, NT + t:NT + t + 1],
                )
                d1 = du.tile([P, K], BF16, tag="d1", name=f"d1_{t}")
                nc.vector.scalar_tensor_tensor(
                    out=d1[:], in0=tsg[:, 0:K], scalar=0.5, in1=prt,
                    op0=OP.mult, op1=OP.mult,
                    accum_out=v_sb[:, NT + t:NT + t + 1],
                )
                d2 = du.tile([P, K], BF16, tag="d2", name=f"d2_{t}")
                nc.vector.scalar_tensor_tensor(
                    out=d2[:], in0=tsg[:, 0:K], scalar=0.5, in1=lv[:],
                    op0=OP.mult, op1=OP.mult,
                    accum_out=v_sb[:, t:t + 1],
                )
                d3 = du.tile([P, K], BF16, tag="d3", name=f"d3_{t}")
                nc.vector.tensor_scalar(
                    d3[:], prt, 1.0, None, OP.mult, OP.add,
                    accum_out=v_sb[:, 2 * NT + t:2 * NT + t + 1],
                )
                nc.gpsimd.tensor_copy(out=dc_sb[:, t:t + 1], in_=tsg[:, K:K + 1])

            nc.sync.dma_start(out=h_ops.ap(), in_=ps_sb[:])
            nc.sync.dma_start(out=h_ov.ap(), in_=v_sb[:])
            nc.sync.dma_start(out=h_ouc.ap(), in_=uc_sb[:])
            nc.sync.dma_start(out=h_od.ap(), in_=dc_sb[:])

    nc.compile()
    return nc


def plan_inputs(pred, teacher, weight, label):
    """Bucket rows by label, assign classes to cores (LPT), build per-core inputs."""
    pred = np.asarray(pred)
    teacher = np.asarray(teacher)
    weight = np.asarray(weight)
    lab = np.asarray(label).astype(np.int64)
    B = pred.shape[0]

    counts = np.bincount(lab, minlength=K)
    present = np.nonzero(counts)[0]
    order = present[np.argsort(-counts[present], kind="stable")]
    core_cls = [[] for _ in range(NCORES)]
    core_rows = [0] * NCORES
    for c in order:
        elig = [i for i in range(NCORES) if len(core_cls[i]) < P]
        i = min(elig, key=lambda j: (core_rows[j], len(core_cls[j])))
        core_cls[i].append(int(c))
        core_rows[i] += int(counts[c])
    NT = max(2, -(-max(core_rows) // P))
    NT += NT % 2  # even tile count (teacher stream is split in halves)
    BP = NT * P

    order_by_lab = np.argsort(lab, kind="stable")
    starts = np.zeros(K + 1, np.int64)
    np.cumsum(counts, out=starts[1:])

    wtT_bf = np.ascontiguousarray(weight.T).astype(NPFP8)  # [D, K]
    # packed layout: partition p's row is the concat over chunks a of
    # wtT[a*128 + p, :] -> each DMA is 128 fat contiguous descriptors
    wt_pack = np.ascontiguousarray(
        wtT_bf.reshape(D // P, P, K).transpose(1, 0, 2).reshape(P, (D // P) * K))

    def pack_rows(x2d):
        nt = x2d.shape[0] // P
        return np.ascontiguousarray(
            x2d.reshape(nt, P, -1).transpose(1, 0, 2).reshape(P, -1))

    in_maps, meta = [], []
    for ci in range(NCORES):
        cls = core_cls[ci] or [int(present[0])]  # degenerate: <8 distinct labels
        rows = (np.concatenate([order_by_lab[starts[c]:starts[c + 1]] for c in cls])
                if core_cls[ci] else np.zeros(0, np.int64))
        n = len(rows)
        assert n <= BP
        slot = (np.concatenate(
            [np.full(int(counts[c]), k, np.int32) for k, c in enumerate(cls)])
            if n else np.zeros(0, np.int32))

        predb = np.zeros((BP, K), NPFP8)
        predb[:n] = pred[rows].astype(NPFP8)
        teab = np.zeros((BP, K), NPFP8)
        teab[:n] = teacher[rows].astype(NPFP8)

        ridx = np.zeros((P, NT), np.int32)
        tlv = np.zeros((P, NT), np.float32)
        j = np.arange(n)
        ridx[j % P, j // P] = slot
        tlv[j % P, j // P] = teacher[rows, lab[rows]]
        plv = pred[rows, lab[rows]].astype(np.float64)
        maxp = np.abs(pred[rows]).max(axis=1).astype(np.float64)

        cls_pad = np.asarray(cls + [cls[0]] * (P - len(cls)), np.int64)
        wl = np.ascontiguousarray(wtT_bf[:, cls_pad])  # [D, P]
        wl_pack = np.ascontiguousarray(
            wl.reshape(D // P, P, P).transpose(1, 0, 2).reshape(P, (D // P) * P))
        d1h = np.zeros((P, K), NPFP8)
        d1h[np.arange(P), cls_pad] = NPFP8(1.0)

        in_maps.append({
            "wt": wt_pack, "wl": wl_pack,
            "predb": np.ascontiguousarray(
                np.concatenate([pack_rows(predb), d1h], axis=1)),
            "teab": pack_rows(teab),
            "ridx": ridx, "tlv": tlv,
        })
        meta.append({"n": n, "plv": plv, "slot": slot, "tlv64": tlv.astype(np.float64),
                     "maxp": maxp})

    assert sum(m["n"] for m in meta) == B
    return {"NT": NT, "B": B, "in_maps": in_maps, "meta": meta}


FAST_KEYS = ("wt", "wl", "predb", "teab")


def finish_fast(plan, results):
    """Host combine for the fast path. Returns (loss, error_bound)."""
    NT = plan["NT"]
    total = 0.0
    bound = 0.0
    for ci in range(NCORES):
        r, m = results[ci], plan["meta"][ci]
        n = m["n"]
        slot = m["slot"]

        def col(arr, comp):
            return arr[:, comp * NT:(comp + 1) * NT].astype(np.float64).T.reshape(-1)[:n]

        zt, zp = col(r["o_z"], 0), col(r["o_z"], 1)
        s = r["o_sd"][:, 0:NT].astype(np.float64).T.reshape(-1)[:n]
        dtab = r["o_sd"][:, NT].astype(np.float64)
        d = dtab[slot]
        tlv = m["tlv64"].T.reshape(-1)[:n]
        pl = m["plv"][:n]
        maxp = m["maxp"][:n]

        conf = np.exp(tlv / TEMP) / zt
        u2 = (1.0 - conf) / (2.0 * (K - 1))
        lnu2 = np.log(u2)
        eps = np.maximum(1.0 - d, 0.0)
        vb = 0.5 * conf + 0.5 * d

        H = (K - 1) * u2 * lnu2 + 0.5 * eps + 0.5 * lnu2 * eps + vb * np.log(vb)
        E = u2 * s + (vb - u2) * pl
        total += float(np.sum(H - E / TEMP + np.log(zp)))

        # rigorous remainder bounds, using f32 resolution slack on d
        epsr = eps + 2e-7
        b = (
            0.5 * epsr * maxp / TEMP          # dropped E1h
            + epsr * epsr / (8.0 * u2)        # Slv second order
            + epsr * epsr / (4.0 * u2) * 0.5  # A second order
            + 0.5 * np.abs(lnu2) * 2e-7       # d resolution in A term
        )
        bound += float(np.sum(b))
    loss = (TEMP * TEMP) * total / plan["B"]
    err = (TEMP * TEMP) * bound / plan["B"]
    return np.array(loss, dtype=np.float32), err


def finish_full(plan, results):
    """Host combine for the full path (float64)."""
    NT = plan["NT"]
    total = 0.0
    for ci in range(NCORES):
        r, m = results[ci], plan["meta"][ci]
        n = m["n"]

        def col(arr, comp):
            return arr[:, comp * NT:(comp + 1) * NT].astype(np.float64).T.reshape(-1)[:n]

        zp, slv = col(r["o_ps"], 0), col(r["o_ps"], 1)
        a, e1h, s = col(r["o_v"], 0), col(r["o_v"], 1), col(r["o_v"], 2)
        u2, conf = col(r["o_uc"], 0), col(r["o_uc"], 1)
        d = col(r["o_d"], 0)
        pl = m["plv"][:n]

        vb = 0.5 * conf + 0.5 * d
        H = u2 * slv + a - u2 * np.log(u2) + vb * np.log(vb)
        E = u2 * s + e1h + (vb - u2) * pl
        total += float(np.sum(H - E / TEMP + np.log(zp)))
    loss = (TEMP * TEMP) * total / plan["B"]
    return np.array(loss, dtype=np.float32)


_NC_CACHE = {}


def get_nc(NT: int, kind: str = "fast"):
    key = (kind, NT)
    if key not in _NC_CACHE:
        _NC_CACHE[key] = (build_nc_fast if kind == "fast" else build_nc_full)(NT)
    return _NC_CACHE[key]


def kernel(pred, teacher, weight, label):
    plan = plan_inputs(pred, teacher, weight, label)
    nc = get_nc(plan["NT"], "fast")
    fast_maps = [{k: im[k] for k in FAST_KEYS} for im in plan["in_maps"]]
    res = run_bass_kernel_spmd(nc, fast_maps, core_ids=list(range(NCORES)))
    loss, err = finish_fast(plan, res.results)
    if err <= GUARD_ABS:
        return loss
    # Data violates the diag-dominance bound: run the full on-device kernel.
    nc = get_nc(plan["NT"], "full")
    res = run_bass_kernel_spmd(nc, plan["in_maps"], core_ids=list(range(NCORES)))
    return finish_full(plan, res.results)
